# revision 10
# baseline (speedup 1.0000x reference)
"""Gaussian KDE (brute-force, bandwidth^2 = 1) on 8 Trainium2 NeuronCores.

Math:
    out_i = log( sum_j w_j * exp(-||x_i - y_j||^2/2) ) - (d/2) log(2pi) - log(sum_j w_j)
          = log( sum_j exp(x_i . y_j + b_j) ) - ||x_i||^2/2 - consts
    with b_j = log(w_j) - ||y_j||^2/2.

Queries sharded 8 ways (512/core). Per core the train axis is split:

  A-path (trains [N_D, 65536), queries-on-partitions, 4 tiles of 128):
    K=34 bf16 matmuls (stationary = query tile, moving = train slices,
    pre-scaled so PSUM holds C1*s + C2B/32) into asymmetric ping/pong
    PSUM tiles (1536 = banks 0-2, 1024 = banks 3-4); ACT table-exp in
    place with scale=1/C1, fused free-dim sum via accum_out.

  D-path (trains [0, N_D), trains-on-partitions, blocks of 128 trains
  x 512 queries, two strips rg=0/64):
    K=34 matmul (stationary = train block incl. bias rows, moving =
    query columns) -> PSUM banks 6/7; DVE tensor_scalar (mult 32, max 0)
    -> int16 = bf16 bits of exp (Schraudolph); PE ones-matmul sums the
    128 trains of each E tile, accumulating all blocks into a persistent
    [1, 512] f32 accumulator in PSUM bank 5 (esum batches of 8 keep the
    PE stream dense). Removes the baseline's DVE tensor_reduce entirely.

  Final: per-qt reduce of ACT partials + esum row; combined on host:
  out = log(A + S) - per-query const. Warm-up ones-matmuls at kernel
  start keep the PE HAM un-throttled through the initial DMA window.
"""

import numpy as np
import ml_dtypes

_Q, _N, _DIM = 4096, 65536, 32
_NCORES = 8
_QSHARD = _Q // _NCORES          # 512 queries per core
_K = 34                          # 32 dims + bias hi/lo rows
_QT = 4                          # query tiles per core

_BF16 = ml_dtypes.bfloat16
_C1 = 4.0 / float(np.log(2.0))


def _c2b():
    f = (np.arange(100000, dtype=np.float64) + 0.5) / 100000.0
    m0 = np.mean((1.0 + f) * 2.0 ** (-f))
    m1 = np.mean(2.0 ** (-f))
    delta = (m0 - 1.0) / m1
    return float(127 * 128 - delta * 128)


_C2B = _c2b()

# train split: D-path trains (multiple of 256), A-path gets the rest
_ND = 30208
_ND2 = _ND // 2                  # per strip
_NA = _N - _ND                   # 35328 per query tile
_NBLK = _ND // 128               # 236 D blocks

# A-unit bankset capacities: bankset 0 = 1536 (banks 0-2),
# bankset 1 = 1024 (banks 3-4); unit width = min(cap, remaining)
_AW_CAP = (1536, 1024)

# measured per-unit engine costs (ns) for the credit scheduler
_ACT_NS = {1536: 1760.0, 1024: 1330.0, 512: 950.0}
_TS_NS = 725.0
_EB = 8                          # esum batch
_WARMUP_MM = 88

_prog_cache: dict = {}


def _a_units(parity):
    """Width list for a qt whose round-0 bankset is `parity`.

    qt q gets bankset (q + r) & 1 at round r; widths are
    min(cap[bankset], remaining)."""
    ws = []
    rem = _NA
    r = 0
    while rem > 0:
        w = min(_AW_CAP[(parity + r) & 1], rem)
        ws.append(w)
        rem -= w
        r += 1
    return ws


def _build_program():
    import concourse.bass as bass
    import concourse.tile as tile
    from concourse import bacc, mybir

    f32 = mybir.dt.float32
    bf16 = mybir.dt.bfloat16
    i16 = mybir.dt.int16

    nc = bacc.Bacc("TRN2", target_bir_lowering=False, debug=False,
                   num_devices=_NCORES)

    ya_d = nc.dram_tensor("ya", [_K, _NA], bf16, kind="ExternalInput")
    yd_d = nc.dram_tensor("ydh", [2 * _K, _ND2], bf16, kind="ExternalInput")
    x_d = nc.dram_tensor("xext", [_K, _QSHARD], bf16, kind="ExternalInput")
    outa_d = nc.dram_tensor("outa", [128, _QT], f32, kind="ExternalOutput")
    outs_d = nc.dram_tensor("outs", [1, _QSHARD], f32, kind="ExternalOutput")

    au_by_parity = (_a_units(0), _a_units(1))
    n_au = len(au_by_parity[0])
    assert len(au_by_parity[1]) == n_au, "A unit counts differ by parity"

    with tile.TileContext(nc) as tc:
        with (
            tc.tile_pool(name="const", bufs=1) as cpool,
            tc.tile_pool(name="q16", bufs=24) as qpool,
            tc.tile_pool(name="small", bufs=2) as spool,
            tc.tile_pool(name="ps", bufs=1, space="PSUM") as ppool,
        ):
            # x stationary/moving copies at partitions 0-33 and 64-97
            xsb = cpool.tile([128, _QSHARD], bf16)
            nc.sync.dma_start(xsb[0:_K, :], x_d[:])
            nc.sync.dma_start(xsb[64:64 + _K, :], x_d[:])
            bias_sb = cpool.tile([128, 1], f32)
            nc.vector.memset(bias_sb[:], -_C2B / (32.0 * _C1))
            ones = cpool.tile([128, 1], bf16)
            nc.vector.memset(ones[:], 1.0)
            wrm = cpool.tile([128, 512], bf16)
            nc.vector.memset(wrm[:], 0.0)

            # y SBUF: A strips + D strips at partitions 0-33 / 64-97
            ysa = cpool.tile([128, _NA], bf16)
            ysd = cpool.tile([128, _ND2], bf16)

            # DMA plan: pieces of 2048 cols, emitted in deadline order so
            # each queue's serial delivery stays ahead of both consumers.
            # A col c consumed at ~4.83c ns; D strip col c at ~11.33c ns.
            PW = 2048
            pieces = []          # (deadline, kind, col, width)
            for c in range(0, _NA, PW):
                pieces.append((4.83 * c + 3000.0, 'a', c, min(PW, _NA - c)))
            for c in range(0, _ND2, PW):
                pieces.append((11.33 * c, 'd', c, min(PW, _ND2 - c)))
            pieces.sort()
            for _, kind, c, w in pieces:
                if kind == 'a':
                    nc.sync.dma_start(ysa[0:_K, c:c + w], ya_d[:, c:c + w])
                    nc.scalar.dma_start(ysa[64:64 + _K, c:c + w],
                                        ya_d[:, c:c + w])
                else:
                    nc.sync.dma_start(ysd[0:_K, c:c + w], yd_d[0:_K, c:c + w])
                    nc.scalar.dma_start(ysd[64:64 + _K, c:c + w],
                                        yd_d[_K:2 * _K, c:c + w])

            NCQ = n_au + 2
            sall = cpool.tile([128, NCQ * _QT], f32)

            # PSUM: A ping [0:1536] banks 0-2, A pong [1536:2560] banks
            # 3-4, esum [2560:3072] bank 5, D ping/pong banks 6/7
            pp = ppool.tile([128, 8 * 512], f32)
            ES0 = 2560
            D0 = 3072

            # PE warm-up: harmless ones-matmuls into D banks
            for i in range(_WARMUP_MM):
                nc.tensor.matmul(
                    out=pp[0:1, D0 + 512 * (i & 1):D0 + 512 * (i & 1) + 512],
                    lhsT=ones[:, 0:1], rhs=wrm[:],
                    start=True, stop=True)

            rg_par = [0]

            def score_mm_a(qt, dst, t0, width):
                for j in range(0, width, 512):
                    w = min(512, width - j)
                    rg = 64 * (rg_par[0] & 1)
                    rg_par[0] += 1
                    nc.tensor.matmul(
                        out=pp[:, dst + j: dst + j + w],
                        lhsT=xsb[rg:rg + _K, qt * 128:(qt + 1) * 128],
                        rhs=ysa[rg:rg + _K, t0 + j: t0 + j + w],
                        start=True, stop=True,
                        tile_position=(rg, 0),
                    )

            # scheduler state
            acur = [0] * _QT         # next train offset within A region
            aidx = [0] * _QT         # next unit index
            pcol = [0] * _QT
            es_first = [True]
            pend_e = []              # E tiles awaiting esum

            def flush_esums():
                for q16 in pend_e:
                    nc.tensor.matmul(
                        out=pp[0:1, ES0:ES0 + _QSHARD],
                        lhsT=ones[:, 0:1],
                        rhs=q16[:].bitcast(bf16),
                        start=es_first[0], stop=False,
                        skip_group_check=not es_first[0],
                    )
                    es_first[0] = False
                pend_e.clear()

            def emit_a(qt, bankset):
                w = au_by_parity[qt & 1][aidx[qt]]
                assert w <= _AW_CAP[bankset], (qt, bankset, w)
                dst = 0 if bankset == 0 else 1536
                t0 = acur[qt]
                score_mm_a(qt, dst, t0, w)
                c = qt * NCQ + pcol[qt]
                pcol[qt] += 1
                nc.scalar.activation(
                    pp[:, dst:dst + w], pp[:, dst:dst + w],
                    mybir.ActivationFunctionType.Exp,
                    bias=bias_sb[:], scale=1.0 / _C1,
                    accum_out=sall[:, c:c + 1])
                acur[qt] += w
                aidx[qt] += 1

            dblk = [0]

            def emit_d():
                b = dblk[0]
                dblk[0] += 1
                h = b & 1
                c = b >> 1
                rg = 64 * h
                dst = D0 + 512 * h
                nc.tensor.matmul(
                    out=pp[:, dst:dst + _QSHARD],
                    lhsT=ysd[rg:rg + _K, c * 128:(c + 1) * 128],
                    rhs=xsb[rg:rg + _K, :],
                    start=True, stop=True,
                    tile_position=(rg, 0),
                )
                q16 = qpool.tile([128, _QSHARD], i16)
                nc.vector.tensor_scalar(
                    q16[:], pp[:, dst:dst + _QSHARD], 32.0, 0.0,
                    mybir.AluOpType.mult, mybir.AluOpType.max)
                pend_e.append(q16)
                if len(pend_e) >= _EB:
                    flush_esums()

            # A emission order: strict bankset alternation; qt order
            # swaps each round so every qt alternates banksets too.
            a_order = []
            for r in range(n_au):
                qts = (0, 1, 2, 3) if (r & 1) == 0 else (1, 0, 3, 2)
                for j, q in enumerate(qts):
                    a_order.append(q)
            a_emitted = [0]

            ta, td = 0.0, -1500.0
            total_a = n_au * _QT
            while a_emitted[0] < total_a or dblk[0] < _NBLK:
                do_a = (dblk[0] >= _NBLK
                        or (a_emitted[0] < total_a and ta <= td))
                if do_a:
                    e = a_emitted[0]
                    a_emitted[0] += 1
                    qt = a_order[e]
                    bankset = e & 1
                    w = au_by_parity[qt & 1][aidx[qt]]
                    ta += _ACT_NS.get(w, _ACT_NS[512])
                    emit_a(qt, bankset)
                else:
                    emit_d()
                    td += _TS_NS
            flush_esums()
            # close the esum accumulation group
            nc.tensor.matmul(
                out=pp[0:1, ES0:ES0 + _QSHARD],
                lhsT=ones[:, 0:1], rhs=wrm[:],
                start=False, stop=True, skip_group_check=True)

            fin = spool.tile([128, _QT], f32)
            for qt in range(_QT):
                nc.vector.tensor_reduce(
                    fin[:, qt:qt + 1], sall[:, qt * NCQ:qt * NCQ + pcol[qt]],
                    axis=mybir.AxisListType.X, op=mybir.AluOpType.add)
            nc.sync.dma_start(outa_d[:], fin[:])
            ssb = spool.tile([1, _QSHARD], f32)
            nc.vector.tensor_copy(ssb[:], pp[0:1, ES0:ES0 + _QSHARD])
            nc.sync.dma_start(outs_d[:], ssb[:])

    nc.compile()
    return nc


def _get_program():
    if "p" not in _prog_cache:
        _prog_cache["p"] = _build_program()
    return _prog_cache["p"]


def _prep_inputs(X, X_train, sample_weight):
    X = np.ascontiguousarray(np.asarray(X, dtype=np.float32))
    Y = np.ascontiguousarray(np.asarray(X_train, dtype=np.float32))
    w = np.ascontiguousarray(np.asarray(sample_weight, dtype=np.float32))

    w64 = w.astype(np.float64)
    b64 = np.log(np.maximum(w64, 1e-300)) - 0.5 * np.sum(
        Y.astype(np.float64) ** 2, axis=1)
    b64 = np.clip(b64, -35.0, None)
    cb64 = (_C1 * b64 + _C2B / 32.0) / 4.0
    bhi = cb64.astype(np.float32).astype(_BF16)
    blo = (cb64 - bhi.astype(np.float64)).astype(np.float32).astype(_BF16)

    yt = Y.astype(_BF16).T           # [32, N]

    # A region: trains [_ND, N)
    ya = np.empty((_K, _NA), dtype=_BF16)
    ya[0:32] = yt[:, _ND:]
    ya[32] = bhi[_ND:]
    ya[33] = blo[_ND:]

    # D region: trains [0, _ND) in two halves
    ydh = np.empty((2 * _K, _ND2), dtype=_BF16)
    ydh[0:32] = yt[:, 0:_ND2]
    ydh[32] = bhi[0:_ND2]
    ydh[33] = blo[0:_ND2]
    ydh[_K:_K + 32] = yt[:, _ND2:_ND]
    ydh[_K + 32] = bhi[_ND2:_ND]
    ydh[_K + 33] = blo[_ND2:_ND]

    const = 0.5 * _DIM * np.log(2.0 * np.pi) + np.log(np.sum(w64))
    xsq = np.sum(X.astype(np.float64) ** 2, axis=1)
    dv_all = (0.5 * xsq + const)

    in_maps = []
    dvs = []
    for c in range(_NCORES):
        sl = slice(c * _QSHARD, (c + 1) * _QSHARD)
        xq = X[sl]
        xext = np.empty((_K, _QSHARD), dtype=_BF16)
        xext[0:32] = (_C1 * xq.astype(np.float64)).astype(_BF16).T
        xext[32] = np.full(_QSHARD, 4.0, dtype=_BF16)
        xext[33] = np.full(_QSHARD, 4.0, dtype=_BF16)
        in_maps.append({"ya": ya, "ydh": ydh, "xext": xext})
        dvs.append(dv_all[sl])
    return in_maps, dvs


def _gather(results, dvs):
    out = np.empty(_Q, dtype=np.float32)
    for c in range(_NCORES):
        ta = results[c]["outa"].T.reshape(_QSHARD).astype(np.float64)
        ts = results[c]["outs"][0].astype(np.float64)
        out[c * _QSHARD:(c + 1) * _QSHARD] = np.log(ta + ts) - dvs[c]
    return out


def kernel(X, X_train, sample_weight, _want_timing=False):
    from concourse.bass_utils import run_bass_kernel_spmd

    nc = _get_program()
    in_maps, dvs = _prep_inputs(X, X_train, sample_weight)
    kres = run_bass_kernel_spmd(
        nc, in_maps, core_ids=list(range(_NCORES)),
        trace=bool(_want_timing),
    )
    out = _gather(kres.results, dvs)
    if _want_timing:
        return out, kres
    return out


# revision 11
# speedup vs baseline: 1.2669x; 1.2669x over previous
"""Gaussian KDE (brute-force, bandwidth^2 = 1) on 8 Trainium2 NeuronCores.

Math:
    out_i = log( sum_j w_j * exp(-||x_i - y_j||^2/2) ) - (d/2) log(2pi) - log(sum_j w_j)
          = log( sum_j exp(x_i . y_j + b_j) ) - ||x_i||^2/2 - consts
    with b_j = log(w_j) - ||y_j||^2/2.

Queries sharded 8 ways (512/core). Per core the train axis is split:

  A-path (trains [N_D, 65536), queries-on-partitions, 4 tiles of 128):
    K=34 bf16 matmuls (stationary = query tile, moving = train slices,
    pre-scaled so PSUM holds C1*s + C2B/32) into asymmetric ping/pong
    PSUM tiles (1536 = banks 0-2, 1024 = banks 3-4); ACT table-exp in
    place with scale=1/C1, fused free-dim sum via accum_out.

  D-path (trains [0, N_D), trains-on-partitions, blocks of 128 trains
  x 512 queries, two strips rg=0/64):
    K=34 matmul (stationary = train block incl. bias rows, moving =
    query columns) -> PSUM banks 6/7; DVE tensor_scalar (mult 32, max 0)
    -> int16 = bf16 bits of exp (Schraudolph); PE ones-matmul sums the
    128 trains of each E tile, accumulating all blocks into a persistent
    [1, 512] f32 accumulator in PSUM bank 5 (esum batches of 8 keep the
    PE stream dense). Removes the baseline's DVE tensor_reduce entirely.

  Final: per-qt reduce of ACT partials + esum row; combined on host:
  out = log(A + S) - per-query const. Warm-up ones-matmuls at kernel
  start keep the PE HAM un-throttled through the initial DMA window.
"""

import numpy as np
import ml_dtypes

_Q, _N, _DIM = 4096, 65536, 32
_NCORES = 8
_QSHARD = _Q // _NCORES          # 512 queries per core
_K = 34                          # 32 dims + bias hi/lo rows
_QT = 4                          # query tiles per core

_BF16 = ml_dtypes.bfloat16
_C1 = 4.0 / float(np.log(2.0))


def _c2b():
    f = (np.arange(100000, dtype=np.float64) + 0.5) / 100000.0
    m0 = np.mean((1.0 + f) * 2.0 ** (-f))
    m1 = np.mean(2.0 ** (-f))
    delta = (m0 - 1.0) / m1
    return float(127 * 128 - delta * 128)


_C2B = _c2b()

# train split: D-path trains (multiple of 256), A-path gets the rest
_ND = 30208
_ND2 = _ND // 2                  # per strip
_NA = _N - _ND                   # 35328 per query tile
_NBLK = _ND // 128               # 236 D blocks

# A-unit bankset capacities: bankset 0 = 1536 (banks 0-2),
# bankset 1 = 1024 (banks 3-4); unit width = min(cap, remaining)
_AW_CAP = (1536, 1024)

# measured per-unit engine costs (ns) for the credit scheduler
_ACT_NS = {1536: 1760.0, 1024: 1330.0, 512: 950.0}
_TS_NS = 725.0
_EB = 8                          # esum batch
_WARMUP_MM = 88

_prog_cache: dict = {}


def _a_units(parity):
    """Width list for a qt whose round-0 bankset is `parity`.

    qt q gets bankset (q + r) & 1 at round r; widths are
    min(cap[bankset], remaining)."""
    ws = []
    rem = _NA
    r = 0
    while rem > 0:
        w = min(_AW_CAP[(parity + r) & 1], rem)
        ws.append(w)
        rem -= w
        r += 1
    return ws


def _build_program():
    import concourse.bass as bass
    import concourse.tile as tile
    from concourse import bacc, mybir

    f32 = mybir.dt.float32
    bf16 = mybir.dt.bfloat16
    i16 = mybir.dt.int16

    nc = bacc.Bacc("TRN2", target_bir_lowering=False, debug=False,
                   num_devices=_NCORES)

    ya_d = nc.dram_tensor("ya", [_K, _NA], bf16, kind="ExternalInput")
    yd_d = nc.dram_tensor("ydh", [2 * _K, _ND2], bf16, kind="ExternalInput")
    x_d = nc.dram_tensor("xext", [_K, _QSHARD], bf16, kind="ExternalInput")
    outa_d = nc.dram_tensor("outa", [128, _QT], f32, kind="ExternalOutput")
    outs_d = nc.dram_tensor("outs", [1, _QSHARD], f32, kind="ExternalOutput")

    au_by_parity = (_a_units(0), _a_units(1))
    n_au = len(au_by_parity[0])
    assert len(au_by_parity[1]) == n_au, "A unit counts differ by parity"

    with tile.TileContext(nc) as tc:
        with (
            tc.tile_pool(name="const", bufs=1) as cpool,
            tc.tile_pool(name="q16", bufs=24) as qpool,
            tc.tile_pool(name="small", bufs=2) as spool,
            tc.tile_pool(name="ps", bufs=1, space="PSUM") as ppool,
        ):
            # x stationary/moving copies at partitions 0-33 and 64-97
            xsb = cpool.tile([128, _QSHARD], bf16)
            nc.sync.dma_start(xsb[0:_K, :], x_d[:])
            nc.sync.dma_start(xsb[64:64 + _K, :], x_d[:])
            bias_sb = cpool.tile([128, 1], f32)
            nc.vector.memset(bias_sb[:], -_C2B / (32.0 * _C1))
            ones = cpool.tile([128, 1], bf16)
            nc.vector.memset(ones[:], 1.0)
            wrm = cpool.tile([128, 512], bf16)
            nc.vector.memset(wrm[:], 0.0)

            # y SBUF: A strips + D strips at partitions 0-33 / 64-97
            ysa = cpool.tile([128, _NA], bf16)
            ysd = cpool.tile([128, _ND2], bf16)

            # DMA plan: pieces of 2048 cols, emitted in deadline order so
            # each queue's serial delivery stays ahead of both consumers.
            # A col c consumed at ~4.83c ns; D strip col c at ~11.33c ns.
            PW = 2048
            pieces = []          # (deadline, kind, col, width)
            for c in range(0, _NA, PW):
                pieces.append((4.83 * c + 3000.0, 'a', c, min(PW, _NA - c)))
            for c in range(0, _ND2, PW):
                pieces.append((11.33 * c, 'd', c, min(PW, _ND2 - c)))
            pieces.sort()
            for _, kind, c, w in pieces:
                if kind == 'a':
                    nc.sync.dma_start(ysa[0:_K, c:c + w], ya_d[:, c:c + w])
                    nc.sync.dma_start(ysa[64:64 + _K, c:c + w],
                                      ya_d[:, c:c + w])
                else:
                    nc.sync.dma_start(ysd[0:_K, c:c + w], yd_d[0:_K, c:c + w])
                    nc.sync.dma_start(ysd[64:64 + _K, c:c + w],
                                      yd_d[_K:2 * _K, c:c + w])

            NCQ = n_au + 2
            sall = cpool.tile([128, NCQ * _QT], f32)

            # PSUM: A ping [0:1536] banks 0-2, A pong [1536:2560] banks
            # 3-4, esum [2560:3072] bank 5, D ping/pong banks 6/7
            pp = ppool.tile([128, 8 * 512], f32)
            ES0 = 2560
            D0 = 3072

            # PE warm-up: harmless ones-matmuls into D banks
            for i in range(_WARMUP_MM):
                nc.tensor.matmul(
                    out=pp[0:1, D0 + 512 * (i & 1):D0 + 512 * (i & 1) + 512],
                    lhsT=ones[:, 0:1], rhs=wrm[:],
                    start=True, stop=True)

            rg_par = [0]

            def score_mm_a(qt, dst, t0, width):
                for j in range(0, width, 512):
                    w = min(512, width - j)
                    rg = 64 * (rg_par[0] & 1)
                    rg_par[0] += 1
                    nc.tensor.matmul(
                        out=pp[:, dst + j: dst + j + w],
                        lhsT=xsb[rg:rg + _K, qt * 128:(qt + 1) * 128],
                        rhs=ysa[rg:rg + _K, t0 + j: t0 + j + w],
                        start=True, stop=True,
                        tile_position=(rg, 0),
                    )

            # scheduler state
            acur = [0] * _QT         # next train offset within A region
            aidx = [0] * _QT         # next unit index
            pcol = [0] * _QT
            es_first = [True]
            pend_e = []              # E tiles awaiting esum

            def flush_esums():
                for q16 in pend_e:
                    nc.tensor.matmul(
                        out=pp[0:1, ES0:ES0 + _QSHARD],
                        lhsT=ones[:, 0:1],
                        rhs=q16[:].bitcast(bf16),
                        start=es_first[0], stop=False,
                        skip_group_check=not es_first[0],
                    )
                    es_first[0] = False
                pend_e.clear()

            def emit_a(qt, bankset):
                w = au_by_parity[qt & 1][aidx[qt]]
                assert w <= _AW_CAP[bankset], (qt, bankset, w)
                dst = 0 if bankset == 0 else 1536
                t0 = acur[qt]
                score_mm_a(qt, dst, t0, w)
                c = qt * NCQ + pcol[qt]
                pcol[qt] += 1
                nc.scalar.activation(
                    pp[:, dst:dst + w], pp[:, dst:dst + w],
                    mybir.ActivationFunctionType.Exp,
                    bias=bias_sb[:], scale=1.0 / _C1,
                    accum_out=sall[:, c:c + 1])
                acur[qt] += w
                aidx[qt] += 1

            dblk = [0]

            def emit_d():
                b = dblk[0]
                dblk[0] += 1
                h = b & 1
                c = b >> 1
                rg = 64 * h
                dst = D0 + 512 * h
                nc.tensor.matmul(
                    out=pp[:, dst:dst + _QSHARD],
                    lhsT=ysd[rg:rg + _K, c * 128:(c + 1) * 128],
                    rhs=xsb[rg:rg + _K, :],
                    start=True, stop=True,
                    tile_position=(rg, 0),
                )
                q16 = qpool.tile([128, _QSHARD], i16)
                nc.vector.tensor_scalar(
                    q16[:], pp[:, dst:dst + _QSHARD], 32.0, 0.0,
                    mybir.AluOpType.mult, mybir.AluOpType.max)
                pend_e.append(q16)
                if len(pend_e) >= _EB:
                    flush_esums()

            # A emission order: strict bankset alternation; qt order
            # swaps each round so every qt alternates banksets too.
            a_order = []
            for r in range(n_au):
                qts = (0, 1, 2, 3) if (r & 1) == 0 else (1, 0, 3, 2)
                for j, q in enumerate(qts):
                    a_order.append(q)
            a_emitted = [0]

            ta, td = 0.0, -1500.0
            total_a = n_au * _QT
            while a_emitted[0] < total_a or dblk[0] < _NBLK:
                do_a = (dblk[0] >= _NBLK
                        or (a_emitted[0] < total_a and ta <= td))
                if do_a:
                    e = a_emitted[0]
                    a_emitted[0] += 1
                    qt = a_order[e]
                    bankset = e & 1
                    w = au_by_parity[qt & 1][aidx[qt]]
                    ta += _ACT_NS.get(w, _ACT_NS[512])
                    emit_a(qt, bankset)
                else:
                    emit_d()
                    td += _TS_NS
            flush_esums()
            # close the esum accumulation group
            nc.tensor.matmul(
                out=pp[0:1, ES0:ES0 + _QSHARD],
                lhsT=ones[:, 0:1], rhs=wrm[:],
                start=False, stop=True, skip_group_check=True)

            fin = spool.tile([128, _QT], f32)
            for qt in range(_QT):
                nc.vector.tensor_reduce(
                    fin[:, qt:qt + 1], sall[:, qt * NCQ:qt * NCQ + pcol[qt]],
                    axis=mybir.AxisListType.X, op=mybir.AluOpType.add)
            nc.sync.dma_start(outa_d[:], fin[:])
            ssb = spool.tile([1, _QSHARD], f32)
            nc.vector.tensor_copy(ssb[:], pp[0:1, ES0:ES0 + _QSHARD])
            nc.sync.dma_start(outs_d[:], ssb[:])

    nc.compile()
    return nc


def _get_program():
    if "p" not in _prog_cache:
        _prog_cache["p"] = _build_program()
    return _prog_cache["p"]


def _prep_inputs(X, X_train, sample_weight):
    X = np.ascontiguousarray(np.asarray(X, dtype=np.float32))
    Y = np.ascontiguousarray(np.asarray(X_train, dtype=np.float32))
    w = np.ascontiguousarray(np.asarray(sample_weight, dtype=np.float32))

    w64 = w.astype(np.float64)
    b64 = np.log(np.maximum(w64, 1e-300)) - 0.5 * np.sum(
        Y.astype(np.float64) ** 2, axis=1)
    b64 = np.clip(b64, -35.0, None)
    cb64 = (_C1 * b64 + _C2B / 32.0) / 4.0
    bhi = cb64.astype(np.float32).astype(_BF16)
    blo = (cb64 - bhi.astype(np.float64)).astype(np.float32).astype(_BF16)

    yt = Y.astype(_BF16).T           # [32, N]

    # A region: trains [_ND, N)
    ya = np.empty((_K, _NA), dtype=_BF16)
    ya[0:32] = yt[:, _ND:]
    ya[32] = bhi[_ND:]
    ya[33] = blo[_ND:]

    # D region: trains [0, _ND) in two halves
    ydh = np.empty((2 * _K, _ND2), dtype=_BF16)
    ydh[0:32] = yt[:, 0:_ND2]
    ydh[32] = bhi[0:_ND2]
    ydh[33] = blo[0:_ND2]
    ydh[_K:_K + 32] = yt[:, _ND2:_ND]
    ydh[_K + 32] = bhi[_ND2:_ND]
    ydh[_K + 33] = blo[_ND2:_ND]

    const = 0.5 * _DIM * np.log(2.0 * np.pi) + np.log(np.sum(w64))
    xsq = np.sum(X.astype(np.float64) ** 2, axis=1)
    dv_all = (0.5 * xsq + const)

    in_maps = []
    dvs = []
    for c in range(_NCORES):
        sl = slice(c * _QSHARD, (c + 1) * _QSHARD)
        xq = X[sl]
        xext = np.empty((_K, _QSHARD), dtype=_BF16)
        xext[0:32] = (_C1 * xq.astype(np.float64)).astype(_BF16).T
        xext[32] = np.full(_QSHARD, 4.0, dtype=_BF16)
        xext[33] = np.full(_QSHARD, 4.0, dtype=_BF16)
        in_maps.append({"ya": ya, "ydh": ydh, "xext": xext})
        dvs.append(dv_all[sl])
    return in_maps, dvs


def _gather(results, dvs):
    out = np.empty(_Q, dtype=np.float32)
    for c in range(_NCORES):
        ta = results[c]["outa"].T.reshape(_QSHARD).astype(np.float64)
        ts = results[c]["outs"][0].astype(np.float64)
        out[c * _QSHARD:(c + 1) * _QSHARD] = np.log(ta + ts) - dvs[c]
    return out


def kernel(X, X_train, sample_weight, _want_timing=False):
    from concourse.bass_utils import run_bass_kernel_spmd

    nc = _get_program()
    in_maps, dvs = _prep_inputs(X, X_train, sample_weight)
    kres = run_bass_kernel_spmd(
        nc, in_maps, core_ids=list(range(_NCORES)),
        trace=bool(_want_timing),
    )
    out = _gather(kres.results, dvs)
    if _want_timing:
        return out, kres
    return out


# revision 21
# speedup vs baseline: 1.5350x; 1.2116x over previous
"""Gaussian KDE (brute-force, bandwidth^2 = 1) on 8 Trainium2 NeuronCores.

Math:
    out_i = log( sum_j w_j * exp(-||x_i - y_j||^2/2) ) - (d/2) log(2pi) - log(sum_j w_j)
          = log( sum_j exp(x_i . y_j + b_j) ) - ||x_i||^2/2 - consts
    with b_j = log(w_j) - ||y_j||^2/2.

Queries sharded 8 ways (512/core). Per core the train axis is split:

  A-path (trains [N_D, 65536), queries-on-partitions, 4 tiles of 128):
    K=34 bf16 matmuls (stationary = query tile, moving = train slices,
    pre-scaled so PSUM holds C1*s + C2B/32) into asymmetric ping/pong
    PSUM tiles (1536 = banks 0-2, 1024 = banks 3-4); ACT table-exp in
    place with scale=1/C1, fused free-dim sum via accum_out.

  D-path (trains [0, N_D), trains-on-partitions, blocks of 128 trains
  x 512 queries, two strips rg=0/64):
    K=34 matmul (stationary = train block incl. bias rows, moving =
    query columns) -> PSUM banks 6/7; DVE tensor_scalar (mult 32, max 0)
    -> int16 = bf16 bits of exp (Schraudolph); PE ones-matmul sums the
    128 trains of each E tile, accumulating all blocks into a persistent
    [1, 512] f32 accumulator in PSUM bank 5 (esum batches of 8 keep the
    PE stream dense). Removes the baseline's DVE tensor_reduce entirely.

  Final: per-qt reduce of ACT partials + esum row; combined on host:
  out = log(A + S) - per-query const. Warm-up ones-matmuls at kernel
  start keep the PE HAM un-throttled through the initial DMA window.
"""

import numpy as np
import ml_dtypes

_Q, _N, _DIM = 4096, 65536, 32
_NCORES = 8
_QSHARD = _Q // _NCORES          # 512 queries per core
_K = 34                          # 32 dims + bias hi/lo rows
_QT = 4                          # query tiles per core

_BF16 = ml_dtypes.bfloat16
_C1 = 4.0 / float(np.log(2.0))


def _c2b():
    f = (np.arange(100000, dtype=np.float64) + 0.5) / 100000.0
    m0 = np.mean((1.0 + f) * 2.0 ** (-f))
    m1 = np.mean(2.0 ** (-f))
    delta = (m0 - 1.0) / m1
    return float(127 * 128 - delta * 128)


_C2B = _c2b()

# train split: D-path trains (multiple of 256), A-path gets the rest
_ND = 26624
_ND2 = _ND // 2                  # per strip
_NA = _N - _ND                   # per query tile
_NBLK = _ND // 128               # 208 D blocks
_NB1 = 124                       # D1 blocks (PE half-esum reduce)
_NB2 = _NBLK - _NB1              # D2 blocks (DVE TT-add reduce)

# A-unit bankset capacities: bankset 0 = 1536 (banks 0-2),
# bankset 1 = 1024 (banks 3-4); unit width = min(cap, remaining)
_AW_CAP = (1536, 1024)

# measured per-unit engine costs (ns) for the credit scheduler
_ACT_NS = {1536: 1760.0, 1024: 1330.0, 512: 950.0}
_TS_NS = 725.0                   # D1 block DVE cost
_D2_NS = 1184.0                  # D2 block DVE cost (ts + TT-add)
_EB = 8                          # esum batch (tiles)
_WARMUP_MM = 48

_prog_cache: dict = {}


def _a_units(parity):
    """Width list for a qt whose round-0 bankset is `parity`.

    qt q gets bankset (q + r) & 1 at round r; widths are
    min(cap[bankset], remaining)."""
    ws = []
    rem = _NA
    r = 0
    while rem > 0:
        w = min(_AW_CAP[(parity + r) & 1], rem)
        ws.append(w)
        rem -= w
        r += 1
    return ws


def _build_program():
    import concourse.bass as bass
    import concourse.tile as tile
    from concourse import bacc, mybir

    f32 = mybir.dt.float32
    bf16 = mybir.dt.bfloat16
    i16 = mybir.dt.int16

    nc = bacc.Bacc("TRN2", target_bir_lowering=False, debug=False,
                   num_devices=_NCORES)

    ya_d = nc.dram_tensor("ya", [_K, _NA], bf16, kind="ExternalInput")
    yd_d = nc.dram_tensor("ydh", [2 * _K, _ND2], bf16, kind="ExternalInput")
    x_d = nc.dram_tensor("xext", [_K, _QSHARD], bf16, kind="ExternalInput")
    outa_d = nc.dram_tensor("outa", [128, _QT], f32, kind="ExternalOutput")
    outs_d = nc.dram_tensor("outs", [2, _QSHARD], f32, kind="ExternalOutput")

    au_by_parity = (_a_units(0), _a_units(1))
    n_au = len(au_by_parity[0])
    assert len(au_by_parity[1]) == n_au, "A unit counts differ by parity"

    with tile.TileContext(nc) as tc:
        with (
            tc.tile_pool(name="const", bufs=1) as cpool,
            tc.tile_pool(name="q16", bufs=24) as qpool,
            tc.tile_pool(name="small", bufs=2) as spool,
            tc.tile_pool(name="ps", bufs=1, space="PSUM") as ppool,
        ):
            # x stationary/moving copies at partitions 0-33 and 64-97
            xsb = cpool.tile([128, _QSHARD], bf16)
            nc.sync.dma_start(xsb[0:_K, :], x_d[:])
            nc.sync.dma_start(xsb[64:64 + _K, :], x_d[:])
            bias_sb = cpool.tile([128, 1], f32)
            nc.vector.memset(bias_sb[:], -_C2B / (32.0 * _C1))
            ones = cpool.tile([128, 1], bf16)
            nc.vector.memset(ones[:], 1.0)
            wrm = cpool.tile([128, 512], bf16)
            nc.vector.memset(wrm[:], 0.0)

            # y SBUF: A strips + D strips at partitions 0-33 / 64-97
            ysa = cpool.tile([128, _NA], bf16)
            ysd = cpool.tile([128, _ND2], bf16)

            # DMA plan: pieces of 2048 cols, emitted in deadline order so
            # each queue's serial delivery stays ahead of both consumers.
            # A col c consumed at ~4.83c ns; D strip col c at ~11.33c ns.
            PW = 2048
            pieces = []          # (deadline, kind, col, width)
            for c in range(0, _NA, PW):
                pieces.append((4.83 * c + 3000.0, 'a', c, min(PW, _NA - c)))
            for c in range(0, _ND2, PW):
                pieces.append((14.2 * c, 'd', c, min(PW, _ND2 - c)))
            pieces.sort()
            for _, kind, c, w in pieces:
                if kind == 'a':
                    nc.sync.dma_start(ysa[0:_K, c:c + w], ya_d[:, c:c + w])
                    nc.sync.dma_start(ysa[64:64 + _K, c:c + w],
                                      ya_d[:, c:c + w])
                else:
                    nc.sync.dma_start(ysd[0:_K, c:c + w], yd_d[0:_K, c:c + w])
                    nc.sync.dma_start(ysd[64:64 + _K, c:c + w],
                                      yd_d[_K:2 * _K, c:c + w])

            NCQ = n_au + 2
            sall = cpool.tile([128, NCQ * _QT], f32)

            # PSUM: A ping [0:1536] banks 0-2, A pong [1536:2560] banks
            # 3-4, esum [2560:3072] bank 5, D ping/pong banks 6/7
            pp = ppool.tile([128, 8 * 512], f32)
            ES0 = 2560
            D0 = 3072

            # PE warm-up: harmless ones-matmuls into D banks
            for i in range(_WARMUP_MM):
                nc.tensor.matmul(
                    out=pp[0:1, D0 + 512 * (i & 1):D0 + 512 * (i & 1) + 512],
                    lhsT=ones[:, 0:1], rhs=wrm[:],
                    start=True, stop=True)

            rg_par = [0]

            def score_mm_a(qt, dst, t0, width):
                for j in range(0, width, 512):
                    w = min(512, width - j)
                    rg = 64 * (rg_par[0] & 1)
                    rg_par[0] += 1
                    nc.tensor.matmul(
                        out=pp[:, dst + j: dst + j + w],
                        lhsT=xsb[rg:rg + _K, qt * 128:(qt + 1) * 128],
                        rhs=ysa[rg:rg + _K, t0 + j: t0 + j + w],
                        start=True, stop=True,
                        tile_position=(rg, 0),
                    )

            # D2 running sum (bf16, summed over D2 blocks elementwise)
            sacc = cpool.tile([128, _QSHARD], i16)
            nc.vector.memset(sacc[:], 0)

            # scheduler state
            acur = [0] * _QT         # next train offset within A region
            aidx = [0] * _QT         # next unit index
            pcol = [0] * _QT
            es_first = [True]
            pend_e = []              # E tiles awaiting esum

            def esum_pair(rhs_i16):
                """Two paired K=64 half-esums: rows 0-63 -> partition 0,
                rows 64-127 -> partition 32 of the esum bank."""
                rhs = rhs_i16[:].bitcast(bf16)
                nc.tensor.matmul(
                    out=pp[0:1, ES0:ES0 + _QSHARD],
                    lhsT=ones[0:64, 0:1], rhs=rhs[0:64, :],
                    start=es_first[0], stop=False,
                    skip_group_check=not es_first[0],
                    tile_position=(0, 0),
                )
                es_first[0] = False
                nc.tensor.matmul(
                    out=pp[32:33, ES0:ES0 + _QSHARD],
                    lhsT=ones[64:128, 0:1], rhs=rhs[64:128, :],
                    start=False, stop=False,
                    skip_group_check=True,
                    tile_position=(64, 32),
                )

            def flush_esums():
                for q16 in pend_e:
                    esum_pair(q16)
                pend_e.clear()

            def emit_a(qt, bankset):
                w = au_by_parity[qt & 1][aidx[qt]]
                assert w <= _AW_CAP[bankset], (qt, bankset, w)
                dst = 0 if bankset == 0 else 1536
                t0 = acur[qt]
                score_mm_a(qt, dst, t0, w)
                c = qt * NCQ + pcol[qt]
                pcol[qt] += 1
                nc.scalar.activation(
                    pp[:, dst:dst + w], pp[:, dst:dst + w],
                    mybir.ActivationFunctionType.Exp,
                    bias=bias_sb[:], scale=1.0 / _C1,
                    accum_out=sall[:, c:c + 1])
                acur[qt] += w
                aidx[qt] += 1

            dblk = [0]
            d1_left = [_NB1]
            d2_left = [_NB2]

            def emit_d(kind):
                b = dblk[0]
                dblk[0] += 1
                h = b & 1
                c = b >> 1
                rg = 64 * h
                dst = D0 + 512 * h
                nc.tensor.matmul(
                    out=pp[:, dst:dst + _QSHARD],
                    lhsT=ysd[rg:rg + _K, c * 128:(c + 1) * 128],
                    rhs=xsb[rg:rg + _K, :],
                    start=True, stop=True,
                    tile_position=(rg, 0),
                )
                q16 = qpool.tile([128, _QSHARD], i16)
                nc.vector.tensor_scalar(
                    q16[:], pp[:, dst:dst + _QSHARD], 32.0, 0.0,
                    mybir.AluOpType.mult, mybir.AluOpType.max)
                if kind == 1:
                    pend_e.append(q16)
                    if len(pend_e) >= _EB:
                        flush_esums()
                else:
                    nc.vector.tensor_tensor(
                        sacc[:].bitcast(bf16), sacc[:].bitcast(bf16),
                        q16[:].bitcast(bf16), mybir.AluOpType.add)

            # A emission order: strict bankset alternation; qt order
            # swaps each round so every qt alternates banksets too.
            a_order = []
            for r in range(n_au):
                qts = (0, 1, 2, 3) if (r & 1) == 0 else (1, 0, 3, 2)
                for j, q in enumerate(qts):
                    a_order.append(q)
            a_emitted = [0]

            ta, td = 0.0, -1500.0
            total_a = n_au * _QT
            while a_emitted[0] < total_a or dblk[0] < _NBLK:
                do_a = (dblk[0] >= _NBLK
                        or (a_emitted[0] < total_a and ta <= td))
                if do_a:
                    e = a_emitted[0]
                    a_emitted[0] += 1
                    qt = a_order[e]
                    bankset = e & 1
                    w = au_by_parity[qt & 1][aidx[qt]]
                    ta += _ACT_NS.get(w, _ACT_NS[512])
                    emit_a(qt, bankset)
                else:
                    # pick D1 vs D2 to keep their shares even
                    if (d1_left[0] * _NB2 >= d2_left[0] * _NB1
                            and d1_left[0] > 0) or d2_left[0] == 0:
                        emit_d(1)
                        d1_left[0] -= 1
                        td += _TS_NS
                    else:
                        emit_d(2)
                        d2_left[0] -= 1
                        td += _D2_NS
            flush_esums()
            # fold the D2 running sum into the esum accumulators
            esum_pair(sacc)
            # close the esum accumulation groups (adds zeros)
            nc.tensor.matmul(
                out=pp[0:1, ES0:ES0 + _QSHARD],
                lhsT=ones[0:64, 0:1], rhs=wrm[0:64, :],
                start=False, stop=True, skip_group_check=True,
                tile_position=(0, 0))
            nc.tensor.matmul(
                out=pp[32:33, ES0:ES0 + _QSHARD],
                lhsT=ones[64:128, 0:1], rhs=wrm[64:128, :],
                start=False, stop=True, skip_group_check=True,
                tile_position=(64, 32))

            fin = spool.tile([128, _QT], f32)
            for qt in range(_QT):
                nc.vector.tensor_reduce(
                    fin[:, qt:qt + 1], sall[:, qt * NCQ:qt * NCQ + pcol[qt]],
                    axis=mybir.AxisListType.X, op=mybir.AluOpType.add)
            nc.sync.dma_start(outa_d[:], fin[:])
            ssb = spool.tile([33, _QSHARD], f32)
            nc.vector.tensor_copy(ssb[0:1, :], pp[0:1, ES0:ES0 + _QSHARD])
            nc.vector.tensor_copy(ssb[32:33, :], pp[32:33, ES0:ES0 + _QSHARD])
            nc.sync.dma_start(outs_d[0:1, :], ssb[0:1, :])
            nc.sync.dma_start(outs_d[1:2, :], ssb[32:33, :])

    nc.compile()
    return nc


def _get_program():
    if "p" not in _prog_cache:
        _prog_cache["p"] = _build_program()
    return _prog_cache["p"]


def _prep_inputs(X, X_train, sample_weight):
    X = np.ascontiguousarray(np.asarray(X, dtype=np.float32))
    Y = np.ascontiguousarray(np.asarray(X_train, dtype=np.float32))
    w = np.ascontiguousarray(np.asarray(sample_weight, dtype=np.float32))

    w64 = w.astype(np.float64)
    b64 = np.log(np.maximum(w64, 1e-300)) - 0.5 * np.sum(
        Y.astype(np.float64) ** 2, axis=1)
    b64 = np.clip(b64, -35.0, None)
    cb64 = (_C1 * b64 + _C2B / 32.0) / 4.0
    bhi = cb64.astype(np.float32).astype(_BF16)
    blo = (cb64 - bhi.astype(np.float64)).astype(np.float32).astype(_BF16)

    yt = Y.astype(_BF16).T           # [32, N]

    # A region: trains [_ND, N)
    ya = np.empty((_K, _NA), dtype=_BF16)
    ya[0:32] = yt[:, _ND:]
    ya[32] = bhi[_ND:]
    ya[33] = blo[_ND:]

    # D region: trains [0, _ND) in two halves
    ydh = np.empty((2 * _K, _ND2), dtype=_BF16)
    ydh[0:32] = yt[:, 0:_ND2]
    ydh[32] = bhi[0:_ND2]
    ydh[33] = blo[0:_ND2]
    ydh[_K:_K + 32] = yt[:, _ND2:_ND]
    ydh[_K + 32] = bhi[_ND2:_ND]
    ydh[_K + 33] = blo[_ND2:_ND]

    const = 0.5 * _DIM * np.log(2.0 * np.pi) + np.log(np.sum(w64))
    xsq = np.sum(X.astype(np.float64) ** 2, axis=1)
    dv_all = (0.5 * xsq + const)

    in_maps = []
    dvs = []
    for c in range(_NCORES):
        sl = slice(c * _QSHARD, (c + 1) * _QSHARD)
        xq = X[sl]
        xext = np.empty((_K, _QSHARD), dtype=_BF16)
        xext[0:32] = (_C1 * xq.astype(np.float64)).astype(_BF16).T
        xext[32] = np.full(_QSHARD, 4.0, dtype=_BF16)
        xext[33] = np.full(_QSHARD, 4.0, dtype=_BF16)
        in_maps.append({"ya": ya, "ydh": ydh, "xext": xext})
        dvs.append(dv_all[sl])
    return in_maps, dvs


def _gather(results, dvs):
    out = np.empty(_Q, dtype=np.float32)
    for c in range(_NCORES):
        ta = results[c]["outa"].T.reshape(_QSHARD).astype(np.float64)
        ts = results[c]["outs"].astype(np.float64).sum(axis=0)
        out[c * _QSHARD:(c + 1) * _QSHARD] = np.log(ta + ts) - dvs[c]
    return out


def kernel(X, X_train, sample_weight, _want_timing=False):
    from concourse.bass_utils import run_bass_kernel_spmd

    nc = _get_program()
    in_maps, dvs = _prep_inputs(X, X_train, sample_weight)
    kres = run_bass_kernel_spmd(
        nc, in_maps, core_ids=list(range(_NCORES)),
        trace=bool(_want_timing),
    )
    out = _gather(kres.results, dvs)
    if _want_timing:
        return out, kres
    return out


# revision 24
# speedup vs baseline: 1.7478x; 1.1386x over previous
"""Gaussian KDE (brute-force, bandwidth^2 = 1) on 8 Trainium2 NeuronCores.

Math:
    out_i = log( sum_j w_j * exp(-||x_i - y_j||^2/2) ) - (d/2) log(2pi) - log(sum_j w_j)
          = log( sum_j exp(x_i . y_j + b_j) ) - ||x_i||^2/2 - consts
    with b_j = log(w_j) - ||y_j||^2/2.

Queries sharded 8 ways (512/core). Per core the train axis is split:

  A-path (trains [N_D, 65536), queries-on-partitions, 4 tiles of 128):
    K=34 bf16 matmuls (stationary = query tile, moving = train slices,
    pre-scaled so PSUM holds C1*s + C2B/32) into asymmetric ping/pong
    PSUM tiles (1536 = banks 0-2, 1024 = banks 3-4); ACT table-exp in
    place with scale=1/C1, fused free-dim sum via accum_out.

  D-path (trains [0, N_D), trains-on-partitions, blocks of 128 trains
  x 512 queries, two strips rg=0/64):
    K=34 matmul (stationary = train block incl. bias rows, moving =
    query columns) -> PSUM banks 6/7; DVE tensor_scalar (mult 32, max 0)
    -> int16 = bf16 bits of exp (Schraudolph); PE ones-matmul sums the
    128 trains of each E tile, accumulating all blocks into a persistent
    [1, 512] f32 accumulator in PSUM bank 5 (esum batches of 8 keep the
    PE stream dense). Removes the baseline's DVE tensor_reduce entirely.

  Final: per-qt reduce of ACT partials + esum row; combined on host:
  out = log(A + S) - per-query const. Warm-up ones-matmuls at kernel
  start keep the PE HAM un-throttled through the initial DMA window.
"""

import numpy as np
import ml_dtypes

_Q, _N, _DIM = 4096, 65536, 32
_NCORES = 8
_QSHARD = _Q // _NCORES          # 512 queries per core
_K = 34                          # 32 dims + bias hi/lo rows
_QT = 4                          # query tiles per core

_BF16 = ml_dtypes.bfloat16
_C1 = 4.0 / float(np.log(2.0))


def _c2b():
    f = (np.arange(100000, dtype=np.float64) + 0.5) / 100000.0
    m0 = np.mean((1.0 + f) * 2.0 ** (-f))
    m1 = np.mean(2.0 ** (-f))
    delta = (m0 - 1.0) / m1
    return float(127 * 128 - delta * 128)


_C2B = _c2b()

# train split: D-path trains (multiple of 256), A-path gets the rest
_ND = 24832
_ND2 = _ND // 2                  # per strip
_NA = _N - _ND                   # per query tile
_NBLK = _ND // 128               # D blocks
_NB1 = 72                        # D1 blocks (PE half-esum reduce)
_NB2 = _NBLK - _NB1              # D2 blocks (DVE TT-add reduce)

# A-unit bankset capacities: bankset 0 = 1536 (banks 0-2),
# bankset 1 = 1024 (banks 3-4); unit width = min(cap, remaining)
_AW_CAP = (1536, 1024)

# measured per-unit engine costs (ns) for the credit scheduler
def _act_ns(w):
    return 475.0 + 0.834 * w


_TS_NS = 725.0                   # D1 block DVE cost
_D2_NS = 1184.0                  # D2 block DVE cost (ts + TT-add)
_EB = 8                          # esum batch (tiles)
_WARMUP_MM = 48

_prog_cache: dict = {}


def _a_units(parity):
    """Width list for a qt whose round-0 bankset is `parity`.

    qt q gets bankset (q + r) & 1 at round r; widths are
    min(cap[bankset], remaining)."""
    ws = []
    rem = _NA
    r = 0
    while rem > 0:
        w = min(_AW_CAP[(parity + r) & 1], rem)
        ws.append(w)
        rem -= w
        r += 1
    return ws


def _build_program():
    import concourse.bass as bass
    import concourse.tile as tile
    from concourse import bacc, mybir

    f32 = mybir.dt.float32
    bf16 = mybir.dt.bfloat16
    i16 = mybir.dt.int16

    nc = bacc.Bacc("TRN2", target_bir_lowering=False, debug=False,
                   num_devices=_NCORES)

    ya_d = nc.dram_tensor("ya", [_K, _NA], bf16, kind="ExternalInput")
    yd_d = nc.dram_tensor("ydh", [2 * _K, _ND2], bf16, kind="ExternalInput")
    x_d = nc.dram_tensor("xext", [_K, _QSHARD], bf16, kind="ExternalInput")
    outa_d = nc.dram_tensor("outa", [128, _QT], f32, kind="ExternalOutput")
    outs_d = nc.dram_tensor("outs", [2, _QSHARD], f32, kind="ExternalOutput")

    au_by_parity = (_a_units(0), _a_units(1))
    n_au = len(au_by_parity[0])
    assert len(au_by_parity[1]) == n_au, "A unit counts differ by parity"

    with tile.TileContext(nc) as tc:
        with (
            tc.tile_pool(name="const", bufs=1) as cpool,
            tc.tile_pool(name="q16", bufs=24) as qpool,
            tc.tile_pool(name="small", bufs=2) as spool,
            tc.tile_pool(name="ps", bufs=1, space="PSUM") as ppool,
        ):
            # x stationary/moving copies at partitions 0-33 and 64-97
            xsb = cpool.tile([128, _QSHARD], bf16)
            nc.sync.dma_start(xsb[0:_K, :], x_d[:])
            nc.sync.dma_start(xsb[64:64 + _K, :], x_d[:])
            bias_sb = cpool.tile([128, 1], f32)
            nc.vector.memset(bias_sb[:], -_C2B / (32.0 * _C1))
            ones = cpool.tile([128, 1], bf16)
            nc.vector.memset(ones[:], 1.0)
            wrm = cpool.tile([128, 512], bf16)
            nc.vector.memset(wrm[:], 0.0)

            # y SBUF: A strips + D strips at partitions 0-33 / 64-97
            ysa = cpool.tile([128, _NA], bf16)
            ysd = cpool.tile([128, _ND2], bf16)

            # DMA plan: pieces of 2048 cols, emitted in deadline order so
            # each queue's serial delivery stays ahead of both consumers.
            # A col c consumed at ~4.83c ns; D strip col c at ~11.33c ns.
            PW = 2048
            pieces = []          # (deadline, kind, col, width)
            for c in range(0, _NA, PW):
                pieces.append((4.83 * c + 3000.0, 'a', c, min(PW, _NA - c)))
            for c in range(0, _ND2, PW):
                pieces.append((15.9 * c, 'd', c, min(PW, _ND2 - c)))
            pieces.sort()
            for _, kind, c, w in pieces:
                if kind == 'a':
                    nc.sync.dma_start(ysa[0:_K, c:c + w], ya_d[:, c:c + w])
                    nc.sync.dma_start(ysa[64:64 + _K, c:c + w],
                                      ya_d[:, c:c + w])
                else:
                    nc.sync.dma_start(ysd[0:_K, c:c + w], yd_d[0:_K, c:c + w])
                    nc.sync.dma_start(ysd[64:64 + _K, c:c + w],
                                      yd_d[_K:2 * _K, c:c + w])

            NCQ = n_au + 2
            sall = cpool.tile([128, NCQ * _QT], f32)

            # PSUM: A ping [0:1536] banks 0-2, A pong [1536:2560] banks
            # 3-4, esum [2560:3072] bank 5, D ping/pong banks 6/7
            pp = ppool.tile([128, 8 * 512], f32)
            ES0 = 2560
            D0 = 3072

            # PE warm-up: harmless ones-matmuls into D banks
            for i in range(_WARMUP_MM):
                nc.tensor.matmul(
                    out=pp[0:1, D0 + 512 * (i & 1):D0 + 512 * (i & 1) + 512],
                    lhsT=ones[:, 0:1], rhs=wrm[:],
                    start=True, stop=True)

            rg_par = [0]

            def score_mm_a(qt, dst, t0, width):
                for j in range(0, width, 512):
                    w = min(512, width - j)
                    rg = 64 * (rg_par[0] & 1)
                    rg_par[0] += 1
                    nc.tensor.matmul(
                        out=pp[:, dst + j: dst + j + w],
                        lhsT=xsb[rg:rg + _K, qt * 128:(qt + 1) * 128],
                        rhs=ysa[rg:rg + _K, t0 + j: t0 + j + w],
                        start=True, stop=True,
                        tile_position=(rg, 0),
                    )

            # D2 running sum (bf16, summed over D2 blocks elementwise)
            sacc = cpool.tile([128, _QSHARD], i16)
            nc.vector.memset(sacc[:], 0)

            # scheduler state
            acur = [0] * _QT         # next train offset within A region
            aidx = [0] * _QT         # next unit index
            pcol = [0] * _QT
            es_first = [True]
            pend_e = []              # E tiles awaiting esum

            def esum_pair(rhs_i16):
                """Two paired K=64 half-esums: rows 0-63 -> partition 0,
                rows 64-127 -> partition 32 of the esum bank."""
                rhs = rhs_i16[:].bitcast(bf16)
                nc.tensor.matmul(
                    out=pp[0:1, ES0:ES0 + _QSHARD],
                    lhsT=ones[0:64, 0:1], rhs=rhs[0:64, :],
                    start=es_first[0], stop=False,
                    skip_group_check=not es_first[0],
                    tile_position=(0, 0),
                )
                es_first[0] = False
                nc.tensor.matmul(
                    out=pp[32:33, ES0:ES0 + _QSHARD],
                    lhsT=ones[64:128, 0:1], rhs=rhs[64:128, :],
                    start=False, stop=False,
                    skip_group_check=True,
                    tile_position=(64, 32),
                )

            def flush_esums():
                for q16 in pend_e:
                    esum_pair(q16)
                pend_e.clear()

            def emit_a(qt, bankset):
                w = au_by_parity[qt & 1][aidx[qt]]
                assert w <= _AW_CAP[bankset], (qt, bankset, w)
                dst = 0 if bankset == 0 else 1536
                t0 = acur[qt]
                score_mm_a(qt, dst, t0, w)
                c = qt * NCQ + pcol[qt]
                pcol[qt] += 1
                nc.scalar.activation(
                    pp[:, dst:dst + w], pp[:, dst:dst + w],
                    mybir.ActivationFunctionType.Exp,
                    bias=bias_sb[:], scale=1.0 / _C1,
                    accum_out=sall[:, c:c + 1])
                acur[qt] += w
                aidx[qt] += 1

            dblk = [0]
            d1_left = [_NB1]
            d2_left = [_NB2]

            def emit_d(kind):
                b = dblk[0]
                dblk[0] += 1
                h = b & 1
                c = b >> 1
                rg = 64 * h
                dst = D0 + 512 * h
                nc.tensor.matmul(
                    out=pp[:, dst:dst + _QSHARD],
                    lhsT=ysd[rg:rg + _K, c * 128:(c + 1) * 128],
                    rhs=xsb[rg:rg + _K, :],
                    start=True, stop=True,
                    tile_position=(rg, 0),
                )
                q16 = qpool.tile([128, _QSHARD], i16)
                nc.vector.tensor_scalar(
                    q16[:], pp[:, dst:dst + _QSHARD], 32.0, 0.0,
                    mybir.AluOpType.mult, mybir.AluOpType.max)
                if kind == 1:
                    pend_e.append(q16)
                    if len(pend_e) >= _EB:
                        flush_esums()
                else:
                    nc.vector.tensor_tensor(
                        sacc[:].bitcast(bf16), sacc[:].bitcast(bf16),
                        q16[:].bitcast(bf16), mybir.AluOpType.add)

            # A emission order: strict bankset alternation; qt order
            # swaps each round so every qt alternates banksets too.
            a_order = []
            for r in range(n_au):
                qts = (0, 1, 2, 3) if (r & 1) == 0 else (1, 0, 3, 2)
                for j, q in enumerate(qts):
                    a_order.append(q)
            a_emitted = [0]

            ta, td = 0.0, -1500.0
            total_a = n_au * _QT
            while a_emitted[0] < total_a or dblk[0] < _NBLK:
                do_a = (dblk[0] >= _NBLK
                        or (a_emitted[0] < total_a and ta <= td))
                if do_a:
                    e = a_emitted[0]
                    a_emitted[0] += 1
                    qt = a_order[e]
                    bankset = e & 1
                    w = au_by_parity[qt & 1][aidx[qt]]
                    ta += _act_ns(w)
                    emit_a(qt, bankset)
                else:
                    # pick D1 vs D2 to keep their shares even
                    if (d1_left[0] * _NB2 >= d2_left[0] * _NB1
                            and d1_left[0] > 0) or d2_left[0] == 0:
                        emit_d(1)
                        d1_left[0] -= 1
                        td += _TS_NS
                    else:
                        emit_d(2)
                        d2_left[0] -= 1
                        td += _D2_NS
            flush_esums()
            # fold the D2 running sum into the esum accumulators
            esum_pair(sacc)
            # close the esum accumulation groups (adds zeros)
            nc.tensor.matmul(
                out=pp[0:1, ES0:ES0 + _QSHARD],
                lhsT=ones[0:64, 0:1], rhs=wrm[0:64, :],
                start=False, stop=True, skip_group_check=True,
                tile_position=(0, 0))
            nc.tensor.matmul(
                out=pp[32:33, ES0:ES0 + _QSHARD],
                lhsT=ones[64:128, 0:1], rhs=wrm[64:128, :],
                start=False, stop=True, skip_group_check=True,
                tile_position=(64, 32))

            fin = spool.tile([128, _QT], f32)
            for qt in range(_QT):
                nc.vector.tensor_reduce(
                    fin[:, qt:qt + 1], sall[:, qt * NCQ:qt * NCQ + pcol[qt]],
                    axis=mybir.AxisListType.X, op=mybir.AluOpType.add)
            nc.sync.dma_start(outa_d[:], fin[:])
            ssb = spool.tile([33, _QSHARD], f32)
            nc.vector.tensor_copy(ssb[0:1, :], pp[0:1, ES0:ES0 + _QSHARD])
            nc.vector.tensor_copy(ssb[32:33, :], pp[32:33, ES0:ES0 + _QSHARD])
            nc.sync.dma_start(outs_d[0:1, :], ssb[0:1, :])
            nc.sync.dma_start(outs_d[1:2, :], ssb[32:33, :])

    nc.compile()
    return nc


def _get_program():
    if "p" not in _prog_cache:
        _prog_cache["p"] = _build_program()
    return _prog_cache["p"]


def _prep_inputs(X, X_train, sample_weight):
    X = np.ascontiguousarray(np.asarray(X, dtype=np.float32))
    Y = np.ascontiguousarray(np.asarray(X_train, dtype=np.float32))
    w = np.ascontiguousarray(np.asarray(sample_weight, dtype=np.float32))

    w64 = w.astype(np.float64)
    b64 = np.log(np.maximum(w64, 1e-300)) - 0.5 * np.sum(
        Y.astype(np.float64) ** 2, axis=1)
    b64 = np.clip(b64, -35.0, None)
    cb64 = (_C1 * b64 + _C2B / 32.0) / 4.0
    bhi = cb64.astype(np.float32).astype(_BF16)
    blo = (cb64 - bhi.astype(np.float64)).astype(np.float32).astype(_BF16)

    yt = Y.astype(_BF16).T           # [32, N]

    # A region: trains [_ND, N)
    ya = np.empty((_K, _NA), dtype=_BF16)
    ya[0:32] = yt[:, _ND:]
    ya[32] = bhi[_ND:]
    ya[33] = blo[_ND:]

    # D region: trains [0, _ND) in two halves
    ydh = np.empty((2 * _K, _ND2), dtype=_BF16)
    ydh[0:32] = yt[:, 0:_ND2]
    ydh[32] = bhi[0:_ND2]
    ydh[33] = blo[0:_ND2]
    ydh[_K:_K + 32] = yt[:, _ND2:_ND]
    ydh[_K + 32] = bhi[_ND2:_ND]
    ydh[_K + 33] = blo[_ND2:_ND]

    const = 0.5 * _DIM * np.log(2.0 * np.pi) + np.log(np.sum(w64))
    xsq = np.sum(X.astype(np.float64) ** 2, axis=1)
    dv_all = (0.5 * xsq + const)

    in_maps = []
    dvs = []
    for c in range(_NCORES):
        sl = slice(c * _QSHARD, (c + 1) * _QSHARD)
        xq = X[sl]
        xext = np.empty((_K, _QSHARD), dtype=_BF16)
        xext[0:32] = (_C1 * xq.astype(np.float64)).astype(_BF16).T
        xext[32] = np.full(_QSHARD, 4.0, dtype=_BF16)
        xext[33] = np.full(_QSHARD, 4.0, dtype=_BF16)
        in_maps.append({"ya": ya, "ydh": ydh, "xext": xext})
        dvs.append(dv_all[sl])
    return in_maps, dvs


def _gather(results, dvs):
    out = np.empty(_Q, dtype=np.float32)
    for c in range(_NCORES):
        ta = results[c]["outa"].T.reshape(_QSHARD).astype(np.float64)
        ts = results[c]["outs"].astype(np.float64).sum(axis=0)
        out[c * _QSHARD:(c + 1) * _QSHARD] = np.log(ta + ts) - dvs[c]
    return out


def kernel(X, X_train, sample_weight, _want_timing=False):
    from concourse.bass_utils import run_bass_kernel_spmd

    nc = _get_program()
    in_maps, dvs = _prep_inputs(X, X_train, sample_weight)
    kres = run_bass_kernel_spmd(
        nc, in_maps, core_ids=list(range(_NCORES)),
        trace=bool(_want_timing),
    )
    out = _gather(kres.results, dvs)
    if _want_timing:
        return out, kres
    return out


# revision 26
# speedup vs baseline: 1.7917x; 1.0251x over previous
"""Gaussian KDE (brute-force, bandwidth^2 = 1) on 8 Trainium2 NeuronCores.

Math:
    out_i = log( sum_j w_j * exp(-||x_i - y_j||^2/2) ) - (d/2) log(2pi) - log(sum_j w_j)
          = log( sum_j exp(x_i . y_j + b_j) ) - ||x_i||^2/2 - consts
    with b_j = log(w_j) - ||y_j||^2/2.

Queries sharded 8 ways (512/core). Per core the train axis is split:

  A-path (trains [N_D, 65536), queries-on-partitions, 4 tiles of 128):
    K=34 bf16 matmuls (stationary = query tile, moving = train slices,
    pre-scaled so PSUM holds C1*s + C2B/32) into asymmetric ping/pong
    PSUM tiles (1536 = banks 0-2, 1024 = banks 3-4); ACT table-exp in
    place with scale=1/C1, fused free-dim sum via accum_out.

  D-path (trains [0, N_D), trains-on-partitions, blocks of 128 trains
  x 512 queries, two strips rg=0/64):
    K=34 matmul (stationary = train block incl. bias rows, moving =
    query columns) -> PSUM banks 6/7; DVE tensor_scalar (mult 32, max 0)
    -> int16 = bf16 bits of exp (Schraudolph); PE ones-matmul sums the
    128 trains of each E tile, accumulating all blocks into a persistent
    [1, 512] f32 accumulator in PSUM bank 5 (esum batches of 8 keep the
    PE stream dense). Removes the baseline's DVE tensor_reduce entirely.

  Final: per-qt reduce of ACT partials + esum row; combined on host:
  out = log(A + S) - per-query const. Warm-up ones-matmuls at kernel
  start keep the PE HAM un-throttled through the initial DMA window.
"""

import numpy as np
import ml_dtypes

_Q, _N, _DIM = 4096, 65536, 32
_NCORES = 8
_QSHARD = _Q // _NCORES          # 512 queries per core
_K = 34                          # 32 dims + bias hi/lo rows
_QT = 4                          # query tiles per core

_BF16 = ml_dtypes.bfloat16
_C1 = 4.0 / float(np.log(2.0))


def _c2b():
    f = (np.arange(100000, dtype=np.float64) + 0.5) / 100000.0
    m0 = np.mean((1.0 + f) * 2.0 ** (-f))
    m1 = np.mean(2.0 ** (-f))
    delta = (m0 - 1.0) / m1
    return float(127 * 128 - delta * 128)


_C2B = _c2b()

# train split: D-path trains (multiple of 256), A-path gets the rest
_ND = 26624
_ND2 = _ND // 2                  # per strip
_NA = _N - _ND                   # per query tile
_NBLK = _ND // 128               # D blocks
_NB_V = 59                       # D blocks reduced by DVE TT-add
_NB_G = _NBLK - _NB_V            # D blocks reduced by GPSIMD TT-add

# A-unit bankset capacities: bankset 0 = 1536 (banks 0-2),
# bankset 1 = 1024 (banks 3-4); unit width = min(cap, remaining)
_AW_CAP = (1536, 1536)

# measured per-unit engine costs (ns) for the credit scheduler
def _act_ns(w):
    return 475.0 + 0.834 * w


_TS_NS = 725.0                   # DVE cost of a D block's tensor_scalar
_TT_NS = 459.0                   # DVE TT-add cost
_GT_NS = 1200.0                  # GPSIMD TT-add cost (estimate)
_WARMUP_MM = 48

_prog_cache: dict = {}


def _a_units(parity):
    """Width list for a qt whose round-0 bankset is `parity`.

    qt q gets bankset (q + r) & 1 at round r; widths are
    min(cap[bankset], remaining)."""
    ws = []
    rem = _NA
    r = 0
    while rem > 0:
        w = min(_AW_CAP[(parity + r) & 1], rem)
        ws.append(w)
        rem -= w
        r += 1
    return ws


def _build_program():
    import concourse.bass as bass
    import concourse.tile as tile
    from concourse import bacc, mybir

    f32 = mybir.dt.float32
    bf16 = mybir.dt.bfloat16
    i16 = mybir.dt.int16

    nc = bacc.Bacc("TRN2", target_bir_lowering=False, debug=False,
                   num_devices=_NCORES)

    ya_d = nc.dram_tensor("ya", [_K, _NA], bf16, kind="ExternalInput")
    yd_d = nc.dram_tensor("ydh", [2 * _K, _ND2], bf16, kind="ExternalInput")
    x_d = nc.dram_tensor("xext", [_K, _QSHARD], bf16, kind="ExternalInput")
    outa_d = nc.dram_tensor("outa", [128, _QT], f32, kind="ExternalOutput")
    outs_d = nc.dram_tensor("outs", [2, _QSHARD], f32, kind="ExternalOutput")

    au_by_parity = (_a_units(0), _a_units(1))
    n_au = len(au_by_parity[0])
    assert len(au_by_parity[1]) == n_au, "A unit counts differ by parity"

    with tile.TileContext(nc) as tc:
        with (
            tc.tile_pool(name="const", bufs=1) as cpool,
            tc.tile_pool(name="q16", bufs=24) as qpool,
            tc.tile_pool(name="small", bufs=2) as spool,
            tc.tile_pool(name="ps", bufs=1, space="PSUM") as ppool,
        ):
            # x stationary/moving copies at partitions 0-33 and 64-97
            xsb = cpool.tile([128, _QSHARD], bf16)
            nc.sync.dma_start(xsb[0:_K, :], x_d[:])
            nc.sync.dma_start(xsb[64:64 + _K, :], x_d[:])
            bias_sb = cpool.tile([128, 1], f32)
            nc.vector.memset(bias_sb[:], -_C2B / (32.0 * _C1))
            ones = cpool.tile([128, 1], bf16)
            nc.vector.memset(ones[:], 1.0)
            wrm = cpool.tile([128, 512], bf16)
            nc.vector.memset(wrm[:], 0.0)

            # y SBUF: A strips + D strips at partitions 0-33 / 64-97
            ysa = cpool.tile([128, _NA], bf16)
            ysd = cpool.tile([128, _ND2], bf16)

            # DMA plan: pieces of 2048 cols, emitted in deadline order so
            # each queue's serial delivery stays ahead of both consumers.
            # A col c consumed at ~4.83c ns; D strip col c at ~11.33c ns.
            PW = 2048
            pieces = []          # (deadline, kind, col, width)
            for c in range(0, _NA, PW):
                pieces.append((4.83 * c + 3000.0, 'a', c, min(PW, _NA - c)))
            for c in range(0, _ND2, PW):
                pieces.append((13.5 * c, 'd', c, min(PW, _ND2 - c)))
            pieces.sort()
            for _, kind, c, w in pieces:
                if kind == 'a':
                    nc.sync.dma_start(ysa[0:_K, c:c + w], ya_d[:, c:c + w])
                    nc.sync.dma_start(ysa[64:64 + _K, c:c + w],
                                      ya_d[:, c:c + w])
                else:
                    nc.sync.dma_start(ysd[0:_K, c:c + w], yd_d[0:_K, c:c + w])
                    nc.sync.dma_start(ysd[64:64 + _K, c:c + w],
                                      yd_d[_K:2 * _K, c:c + w])

            NCQ = n_au + 2
            sall = cpool.tile([128, NCQ * _QT], f32)

            # PSUM: A ping [0:1536] banks 0-2, A pong [1536:3072] banks
            # 3-5, D ping/pong banks 6/7; bank 0 is reused at the very
            # end for the two final partition-fold matmuls.
            pp = ppool.tile([128, 8 * 512], f32)
            ES0 = 0
            D0 = 3072

            # PE warm-up: harmless ones-matmuls into D banks
            for i in range(_WARMUP_MM):
                nc.tensor.matmul(
                    out=pp[0:1, D0 + 512 * (i & 1):D0 + 512 * (i & 1) + 512],
                    lhsT=ones[:, 0:1], rhs=wrm[:],
                    start=True, stop=True)

            rg_par = [0]

            def score_mm_a(qt, dst, t0, width):
                for j in range(0, width, 512):
                    w = min(512, width - j)
                    rg = 64 * (rg_par[0] & 1)
                    rg_par[0] += 1
                    nc.tensor.matmul(
                        out=pp[:, dst + j: dst + j + w],
                        lhsT=xsb[rg:rg + _K, qt * 128:(qt + 1) * 128],
                        rhs=ysa[rg:rg + _K, t0 + j: t0 + j + w],
                        start=True, stop=True,
                        tile_position=(rg, 0),
                    )

            # running sums over D blocks (bf16 bits), one per engine
            sacc_v = cpool.tile([128, _QSHARD], i16)
            nc.vector.memset(sacc_v[:], 0)
            sacc_g = cpool.tile([128, _QSHARD], i16)
            nc.vector.memset(sacc_g[:], 0)

            # scheduler state
            acur = [0] * _QT         # next train offset within A region
            aidx = [0] * _QT         # next unit index
            pcol = [0] * _QT
            def esum_pair(rhs_i16, first):
                """Two K=64 half-esums: rows 0-63 -> partition 0, rows
                64-127 -> partition 32 of bank 0 (after A-path is done)."""
                rhs = rhs_i16[:].bitcast(bf16)
                nc.tensor.matmul(
                    out=pp[0:1, ES0:ES0 + _QSHARD],
                    lhsT=ones[0:64, 0:1], rhs=rhs[0:64, :],
                    start=first, stop=False,
                    skip_group_check=not first,
                    tile_position=(0, 0),
                )
                nc.tensor.matmul(
                    out=pp[32:33, ES0:ES0 + _QSHARD],
                    lhsT=ones[64:128, 0:1], rhs=rhs[64:128, :],
                    start=False, stop=False,
                    skip_group_check=True,
                    tile_position=(64, 32),
                )

            def emit_a(qt, bankset):
                w = au_by_parity[qt & 1][aidx[qt]]
                assert w <= _AW_CAP[bankset], (qt, bankset, w)
                dst = 0 if bankset == 0 else 1536
                t0 = acur[qt]
                score_mm_a(qt, dst, t0, w)
                c = qt * NCQ + pcol[qt]
                pcol[qt] += 1
                nc.scalar.activation(
                    pp[:, dst:dst + w], pp[:, dst:dst + w],
                    mybir.ActivationFunctionType.Exp,
                    bias=bias_sb[:], scale=1.0 / _C1,
                    accum_out=sall[:, c:c + 1])
                acur[qt] += w
                aidx[qt] += 1

            dblk = [0]
            dv_left = [_NB_V]
            dg_left = [_NB_G]

            def emit_d(kind):
                b = dblk[0]
                dblk[0] += 1
                h = b & 1
                c = b >> 1
                rg = 64 * h
                dst = D0 + 512 * h
                nc.tensor.matmul(
                    out=pp[:, dst:dst + _QSHARD],
                    lhsT=ysd[rg:rg + _K, c * 128:(c + 1) * 128],
                    rhs=xsb[rg:rg + _K, :],
                    start=True, stop=True,
                    tile_position=(rg, 0),
                )
                q16 = qpool.tile([128, _QSHARD], i16)
                nc.vector.tensor_scalar(
                    q16[:], pp[:, dst:dst + _QSHARD], 32.0, 0.0,
                    mybir.AluOpType.mult, mybir.AluOpType.max)
                if kind == 'v':
                    nc.vector.tensor_tensor(
                        sacc_v[:].bitcast(bf16), sacc_v[:].bitcast(bf16),
                        q16[:].bitcast(bf16), mybir.AluOpType.add)
                else:
                    nc.gpsimd.tensor_tensor(
                        sacc_g[:].bitcast(bf16), sacc_g[:].bitcast(bf16),
                        q16[:].bitcast(bf16), mybir.AluOpType.add)

            # A emission order: strict bankset alternation; qt order
            # swaps each round so every qt alternates banksets too.
            a_order = []
            for r in range(n_au):
                qts = (0, 1, 2, 3) if (r & 1) == 0 else (1, 0, 3, 2)
                for j, q in enumerate(qts):
                    a_order.append(q)
            a_emitted = [0]

            ta, td = 0.0, -1500.0
            total_a = n_au * _QT
            while a_emitted[0] < total_a or dblk[0] < _NBLK:
                do_a = (dblk[0] >= _NBLK
                        or (a_emitted[0] < total_a and ta <= td))
                if do_a:
                    e = a_emitted[0]
                    a_emitted[0] += 1
                    qt = a_order[e]
                    bankset = e & 1
                    w = au_by_parity[qt & 1][aidx[qt]]
                    ta += _act_ns(w)
                    emit_a(qt, bankset)
                else:
                    # pick DVE vs GPSIMD reduce to keep shares even
                    if (dg_left[0] * _NB_V >= dv_left[0] * _NB_G
                            and dg_left[0] > 0) or dv_left[0] == 0:
                        emit_d('g')
                        dg_left[0] -= 1
                        td += _TS_NS
                    else:
                        emit_d('v')
                        dv_left[0] -= 1
                        td += _TS_NS + _TT_NS
            # final partition-folds of the two running sums (bank 0 is
            # free once the last A-ping unit has been consumed)
            esum_pair(sacc_v, True)
            esum_pair(sacc_g, False)
            # close the accumulation groups (adds zeros)
            nc.tensor.matmul(
                out=pp[0:1, ES0:ES0 + _QSHARD],
                lhsT=ones[0:64, 0:1], rhs=wrm[0:64, :],
                start=False, stop=True, skip_group_check=True,
                tile_position=(0, 0))
            nc.tensor.matmul(
                out=pp[32:33, ES0:ES0 + _QSHARD],
                lhsT=ones[64:128, 0:1], rhs=wrm[64:128, :],
                start=False, stop=True, skip_group_check=True,
                tile_position=(64, 32))

            fin = spool.tile([128, _QT], f32)
            for qt in range(_QT):
                nc.vector.tensor_reduce(
                    fin[:, qt:qt + 1], sall[:, qt * NCQ:qt * NCQ + pcol[qt]],
                    axis=mybir.AxisListType.X, op=mybir.AluOpType.add)
            nc.sync.dma_start(outa_d[:], fin[:])
            ssb = spool.tile([33, _QSHARD], f32)
            nc.vector.tensor_copy(ssb[0:1, :], pp[0:1, ES0:ES0 + _QSHARD])
            nc.vector.tensor_copy(ssb[32:33, :], pp[32:33, ES0:ES0 + _QSHARD])
            nc.sync.dma_start(outs_d[0:1, :], ssb[0:1, :])
            nc.sync.dma_start(outs_d[1:2, :], ssb[32:33, :])

    nc.compile()
    return nc


def _get_program():
    if "p" not in _prog_cache:
        _prog_cache["p"] = _build_program()
    return _prog_cache["p"]


def _prep_inputs(X, X_train, sample_weight):
    X = np.ascontiguousarray(np.asarray(X, dtype=np.float32))
    Y = np.ascontiguousarray(np.asarray(X_train, dtype=np.float32))
    w = np.ascontiguousarray(np.asarray(sample_weight, dtype=np.float32))

    w64 = w.astype(np.float64)
    b64 = np.log(np.maximum(w64, 1e-300)) - 0.5 * np.sum(
        Y.astype(np.float64) ** 2, axis=1)
    b64 = np.clip(b64, -35.0, None)
    cb64 = (_C1 * b64 + _C2B / 32.0) / 4.0
    bhi = cb64.astype(np.float32).astype(_BF16)
    blo = (cb64 - bhi.astype(np.float64)).astype(np.float32).astype(_BF16)

    yt = Y.astype(_BF16).T           # [32, N]

    # A region: trains [_ND, N)
    ya = np.empty((_K, _NA), dtype=_BF16)
    ya[0:32] = yt[:, _ND:]
    ya[32] = bhi[_ND:]
    ya[33] = blo[_ND:]

    # D region: trains [0, _ND) in two halves
    ydh = np.empty((2 * _K, _ND2), dtype=_BF16)
    ydh[0:32] = yt[:, 0:_ND2]
    ydh[32] = bhi[0:_ND2]
    ydh[33] = blo[0:_ND2]
    ydh[_K:_K + 32] = yt[:, _ND2:_ND]
    ydh[_K + 32] = bhi[_ND2:_ND]
    ydh[_K + 33] = blo[_ND2:_ND]

    const = 0.5 * _DIM * np.log(2.0 * np.pi) + np.log(np.sum(w64))
    xsq = np.sum(X.astype(np.float64) ** 2, axis=1)
    dv_all = (0.5 * xsq + const)

    in_maps = []
    dvs = []
    for c in range(_NCORES):
        sl = slice(c * _QSHARD, (c + 1) * _QSHARD)
        xq = X[sl]
        xext = np.empty((_K, _QSHARD), dtype=_BF16)
        xext[0:32] = (_C1 * xq.astype(np.float64)).astype(_BF16).T
        xext[32] = np.full(_QSHARD, 4.0, dtype=_BF16)
        xext[33] = np.full(_QSHARD, 4.0, dtype=_BF16)
        in_maps.append({"ya": ya, "ydh": ydh, "xext": xext})
        dvs.append(dv_all[sl])
    return in_maps, dvs


def _gather(results, dvs):
    out = np.empty(_Q, dtype=np.float32)
    for c in range(_NCORES):
        ta = results[c]["outa"].T.reshape(_QSHARD).astype(np.float64)
        ts = results[c]["outs"].astype(np.float64).sum(axis=0)
        out[c * _QSHARD:(c + 1) * _QSHARD] = np.log(ta + ts) - dvs[c]
    return out


def kernel(X, X_train, sample_weight, _want_timing=False):
    from concourse.bass_utils import run_bass_kernel_spmd

    nc = _get_program()
    in_maps, dvs = _prep_inputs(X, X_train, sample_weight)
    kres = run_bass_kernel_spmd(
        nc, in_maps, core_ids=list(range(_NCORES)),
        trace=bool(_want_timing),
    )
    out = _gather(kres.results, dvs)
    if _want_timing:
        return out, kres
    return out


# revision 27
# speedup vs baseline: 1.8262x; 1.0193x over previous
"""Gaussian KDE (brute-force, bandwidth^2 = 1) on 8 Trainium2 NeuronCores.

Math:
    out_i = log( sum_j w_j * exp(-||x_i - y_j||^2/2) ) - (d/2) log(2pi) - log(sum_j w_j)
          = log( sum_j exp(x_i . y_j + b_j) ) - ||x_i||^2/2 - consts
    with b_j = log(w_j) - ||y_j||^2/2.

Queries sharded 8 ways (512/core). Per core the train axis is split:

  A-path (trains [N_D, 65536), queries-on-partitions, 4 tiles of 128):
    K=34 bf16 matmuls (stationary = query tile, moving = train slices,
    pre-scaled so PSUM holds C1*s + C2B/32) into asymmetric ping/pong
    PSUM tiles (1536 = banks 0-2, 1024 = banks 3-4); ACT table-exp in
    place with scale=1/C1, fused free-dim sum via accum_out.

  D-path (trains [0, N_D), trains-on-partitions, blocks of 128 trains
  x 512 queries, two strips rg=0/64):
    K=34 matmul (stationary = train block incl. bias rows, moving =
    query columns) -> PSUM banks 6/7; DVE tensor_scalar (mult 32, max 0)
    -> int16 = bf16 bits of exp (Schraudolph); PE ones-matmul sums the
    128 trains of each E tile, accumulating all blocks into a persistent
    [1, 512] f32 accumulator in PSUM bank 5 (esum batches of 8 keep the
    PE stream dense). Removes the baseline's DVE tensor_reduce entirely.

  Final: per-qt reduce of ACT partials + esum row; combined on host:
  out = log(A + S) - per-query const. Warm-up ones-matmuls at kernel
  start keep the PE HAM un-throttled through the initial DMA window.
"""

import numpy as np
import ml_dtypes

_Q, _N, _DIM = 4096, 65536, 32
_NCORES = 8
_QSHARD = _Q // _NCORES          # 512 queries per core
_K = 34                          # 32 dims + bias hi/lo rows
_QT = 4                          # query tiles per core

_BF16 = ml_dtypes.bfloat16
_C1 = 4.0 / float(np.log(2.0))


def _c2b():
    f = (np.arange(100000, dtype=np.float64) + 0.5) / 100000.0
    m0 = np.mean((1.0 + f) * 2.0 ** (-f))
    m1 = np.mean(2.0 ** (-f))
    delta = (m0 - 1.0) / m1
    return float(127 * 128 - delta * 128)


_C2B = _c2b()

# train split: D-path trains (multiple of 256), A-path gets the rest
_ND = 25088
_ND2 = _ND // 2                  # per strip
_NA = _N - _ND                   # per query tile
_NBLK = _ND // 128               # D blocks
_NB_V = 38                       # D blocks reduced by DVE TT-add
_NB_G = _NBLK - _NB_V            # D blocks reduced by GPSIMD TT-add

# A-unit bankset capacities: bankset 0 = 1536 (banks 0-2),
# bankset 1 = 1024 (banks 3-4); unit width = min(cap, remaining)
_AW_CAP = (1536, 1536)

# measured per-unit engine costs (ns) for the credit scheduler
def _act_ns(w):
    return 475.0 + 0.834 * w


_TS_NS = 725.0                   # DVE cost of a D block's tensor_scalar
_TT_NS = 1182.0                  # DVE TT-add cost (degraded by GPSIMD port sharing)
_GT_NS = 1200.0                  # GPSIMD TT-add cost (estimate)
_WARMUP_MM = 16

_prog_cache: dict = {}


def _a_units(parity):
    """Width list for a qt whose round-0 bankset is `parity`.

    qt q gets bankset (q + r) & 1 at round r; widths are
    min(cap[bankset], remaining)."""
    ws = []
    rem = _NA
    r = 0
    while rem > 0:
        w = min(_AW_CAP[(parity + r) & 1], rem)
        ws.append(w)
        rem -= w
        r += 1
    return ws


def _build_program():
    import concourse.bass as bass
    import concourse.tile as tile
    from concourse import bacc, mybir

    f32 = mybir.dt.float32
    bf16 = mybir.dt.bfloat16
    i16 = mybir.dt.int16

    nc = bacc.Bacc("TRN2", target_bir_lowering=False, debug=False,
                   num_devices=_NCORES)

    ya_d = nc.dram_tensor("ya", [_K, _NA], bf16, kind="ExternalInput")
    yd_d = nc.dram_tensor("ydh", [2 * _K, _ND2], bf16, kind="ExternalInput")
    x_d = nc.dram_tensor("xext", [_K, _QSHARD], bf16, kind="ExternalInput")
    outa_d = nc.dram_tensor("outa", [128, _QT], f32, kind="ExternalOutput")
    outs_d = nc.dram_tensor("outs", [2, _QSHARD], f32, kind="ExternalOutput")

    au_by_parity = (_a_units(0), _a_units(1))
    n_au = len(au_by_parity[0])
    assert len(au_by_parity[1]) == n_au, "A unit counts differ by parity"

    with tile.TileContext(nc) as tc:
        with (
            tc.tile_pool(name="const", bufs=1) as cpool,
            tc.tile_pool(name="q16", bufs=24) as qpool,
            tc.tile_pool(name="small", bufs=2) as spool,
            tc.tile_pool(name="ps", bufs=1, space="PSUM") as ppool,
        ):
            # x stationary/moving copies at partitions 0-33 and 64-97
            xsb = cpool.tile([128, _QSHARD], bf16)
            nc.sync.dma_start(xsb[0:_K, :], x_d[:])
            nc.sync.dma_start(xsb[64:64 + _K, :], x_d[:])
            bias_sb = cpool.tile([128, 1], f32)
            nc.vector.memset(bias_sb[:], -_C2B / (32.0 * _C1))
            ones = cpool.tile([128, 1], bf16)
            nc.vector.memset(ones[:], 1.0)
            wrm = cpool.tile([128, 512], bf16)
            nc.vector.memset(wrm[:], 0.0)

            # y SBUF: A strips + D strips at partitions 0-33 / 64-97
            ysa = cpool.tile([128, _NA], bf16)
            ysd = cpool.tile([128, _ND2], bf16)

            # DMA plan: pieces of 2048 cols, emitted in deadline order so
            # each queue's serial delivery stays ahead of both consumers.
            # A col c consumed at ~4.83c ns; D strip col c at ~11.33c ns.
            PW = 2048
            pieces = []          # (deadline, kind, col, width)
            for c in range(0, _NA, PW):
                pieces.append((4.83 * c + 3000.0, 'a', c, min(PW, _NA - c)))
            for c in range(0, _ND2, PW):
                pieces.append((14.7 * c, 'd', c, min(PW, _ND2 - c)))
            pieces.sort()
            for _, kind, c, w in pieces:
                if kind == 'a':
                    nc.sync.dma_start(ysa[0:_K, c:c + w], ya_d[:, c:c + w])
                    nc.sync.dma_start(ysa[64:64 + _K, c:c + w],
                                      ya_d[:, c:c + w])
                else:
                    nc.sync.dma_start(ysd[0:_K, c:c + w], yd_d[0:_K, c:c + w])
                    nc.sync.dma_start(ysd[64:64 + _K, c:c + w],
                                      yd_d[_K:2 * _K, c:c + w])

            NCQ = n_au + 2
            sall = cpool.tile([128, NCQ * _QT], f32)

            # PSUM: A ping [0:1536] banks 0-2, A pong [1536:3072] banks
            # 3-5, D ping/pong banks 6/7; bank 0 is reused at the very
            # end for the two final partition-fold matmuls.
            pp = ppool.tile([128, 8 * 512], f32)
            ES0 = 0
            D0 = 3072

            # PE warm-up: harmless ones-matmuls into D banks
            for i in range(_WARMUP_MM):
                nc.tensor.matmul(
                    out=pp[0:1, D0 + 512 * (i & 1):D0 + 512 * (i & 1) + 512],
                    lhsT=ones[:, 0:1], rhs=wrm[:],
                    start=True, stop=True)

            rg_par = [0]

            def score_mm_a(qt, dst, t0, width):
                for j in range(0, width, 512):
                    w = min(512, width - j)
                    rg = 64 * (rg_par[0] & 1)
                    rg_par[0] += 1
                    nc.tensor.matmul(
                        out=pp[:, dst + j: dst + j + w],
                        lhsT=xsb[rg:rg + _K, qt * 128:(qt + 1) * 128],
                        rhs=ysa[rg:rg + _K, t0 + j: t0 + j + w],
                        start=True, stop=True,
                        tile_position=(rg, 0),
                    )

            # running sums over D blocks (bf16 bits), one per engine
            sacc_v = cpool.tile([128, _QSHARD], i16)
            nc.vector.memset(sacc_v[:], 0)
            sacc_g = cpool.tile([128, _QSHARD], i16)
            nc.vector.memset(sacc_g[:], 0)

            # scheduler state
            acur = [0] * _QT         # next train offset within A region
            aidx = [0] * _QT         # next unit index
            pcol = [0] * _QT
            def esum_pair(rhs_i16, first):
                """Two K=64 half-esums: rows 0-63 -> partition 0, rows
                64-127 -> partition 32 of bank 0 (after A-path is done)."""
                rhs = rhs_i16[:].bitcast(bf16)
                nc.tensor.matmul(
                    out=pp[0:1, ES0:ES0 + _QSHARD],
                    lhsT=ones[0:64, 0:1], rhs=rhs[0:64, :],
                    start=first, stop=False,
                    skip_group_check=not first,
                    tile_position=(0, 0),
                )
                nc.tensor.matmul(
                    out=pp[32:33, ES0:ES0 + _QSHARD],
                    lhsT=ones[64:128, 0:1], rhs=rhs[64:128, :],
                    start=False, stop=False,
                    skip_group_check=True,
                    tile_position=(64, 32),
                )

            def emit_a(qt, bankset):
                w = au_by_parity[qt & 1][aidx[qt]]
                assert w <= _AW_CAP[bankset], (qt, bankset, w)
                dst = 0 if bankset == 0 else 1536
                t0 = acur[qt]
                score_mm_a(qt, dst, t0, w)
                c = qt * NCQ + pcol[qt]
                pcol[qt] += 1
                nc.scalar.activation(
                    pp[:, dst:dst + w], pp[:, dst:dst + w],
                    mybir.ActivationFunctionType.Exp,
                    bias=bias_sb[:], scale=1.0 / _C1,
                    accum_out=sall[:, c:c + 1])
                acur[qt] += w
                aidx[qt] += 1

            dblk = [0]
            dv_left = [_NB_V]
            dg_left = [_NB_G]

            def emit_d(kind):
                b = dblk[0]
                dblk[0] += 1
                h = b & 1
                c = b >> 1
                rg = 64 * h
                dst = D0 + 512 * h
                nc.tensor.matmul(
                    out=pp[:, dst:dst + _QSHARD],
                    lhsT=ysd[rg:rg + _K, c * 128:(c + 1) * 128],
                    rhs=xsb[rg:rg + _K, :],
                    start=True, stop=True,
                    tile_position=(rg, 0),
                )
                q16 = qpool.tile([128, _QSHARD], i16)
                nc.vector.tensor_scalar(
                    q16[:], pp[:, dst:dst + _QSHARD], 32.0, 0.0,
                    mybir.AluOpType.mult, mybir.AluOpType.max)
                if kind == 'v':
                    nc.vector.tensor_tensor(
                        sacc_v[:].bitcast(bf16), sacc_v[:].bitcast(bf16),
                        q16[:].bitcast(bf16), mybir.AluOpType.add)
                else:
                    nc.gpsimd.tensor_tensor(
                        sacc_g[:].bitcast(bf16), sacc_g[:].bitcast(bf16),
                        q16[:].bitcast(bf16), mybir.AluOpType.add)

            # A emission order: strict bankset alternation; qt order
            # swaps each round so every qt alternates banksets too.
            a_order = []
            for r in range(n_au):
                qts = (0, 1, 2, 3) if (r & 1) == 0 else (1, 0, 3, 2)
                for j, q in enumerate(qts):
                    a_order.append(q)
            a_emitted = [0]

            ta, td = 0.0, -1500.0
            total_a = n_au * _QT
            while a_emitted[0] < total_a or dblk[0] < _NBLK:
                do_a = (dblk[0] >= _NBLK
                        or (a_emitted[0] < total_a and ta <= td))
                if do_a:
                    e = a_emitted[0]
                    a_emitted[0] += 1
                    qt = a_order[e]
                    bankset = e & 1
                    w = au_by_parity[qt & 1][aidx[qt]]
                    ta += _act_ns(w)
                    emit_a(qt, bankset)
                else:
                    # pick DVE vs GPSIMD reduce to keep shares even
                    if (dg_left[0] * _NB_V >= dv_left[0] * _NB_G
                            and dg_left[0] > 0) or dv_left[0] == 0:
                        emit_d('g')
                        dg_left[0] -= 1
                        td += _TS_NS
                    else:
                        emit_d('v')
                        dv_left[0] -= 1
                        td += _TS_NS + _TT_NS
            # final partition-folds of the two running sums (bank 0 is
            # free once the last A-ping unit has been consumed)
            esum_pair(sacc_v, True)
            esum_pair(sacc_g, False)
            # close the accumulation groups (adds zeros)
            nc.tensor.matmul(
                out=pp[0:1, ES0:ES0 + _QSHARD],
                lhsT=ones[0:64, 0:1], rhs=wrm[0:64, :],
                start=False, stop=True, skip_group_check=True,
                tile_position=(0, 0))
            nc.tensor.matmul(
                out=pp[32:33, ES0:ES0 + _QSHARD],
                lhsT=ones[64:128, 0:1], rhs=wrm[64:128, :],
                start=False, stop=True, skip_group_check=True,
                tile_position=(64, 32))

            fin = spool.tile([128, _QT], f32)
            for qt in range(_QT):
                nc.vector.tensor_reduce(
                    fin[:, qt:qt + 1], sall[:, qt * NCQ:qt * NCQ + pcol[qt]],
                    axis=mybir.AxisListType.X, op=mybir.AluOpType.add)
            nc.sync.dma_start(outa_d[:], fin[:])
            ssb = spool.tile([33, _QSHARD], f32)
            nc.vector.tensor_copy(ssb[0:1, :], pp[0:1, ES0:ES0 + _QSHARD])
            nc.vector.tensor_copy(ssb[32:33, :], pp[32:33, ES0:ES0 + _QSHARD])
            nc.sync.dma_start(outs_d[0:1, :], ssb[0:1, :])
            nc.sync.dma_start(outs_d[1:2, :], ssb[32:33, :])

    nc.compile()
    return nc


def _get_program():
    if "p" not in _prog_cache:
        _prog_cache["p"] = _build_program()
    return _prog_cache["p"]


def _prep_inputs(X, X_train, sample_weight):
    X = np.ascontiguousarray(np.asarray(X, dtype=np.float32))
    Y = np.ascontiguousarray(np.asarray(X_train, dtype=np.float32))
    w = np.ascontiguousarray(np.asarray(sample_weight, dtype=np.float32))

    w64 = w.astype(np.float64)
    b64 = np.log(np.maximum(w64, 1e-300)) - 0.5 * np.sum(
        Y.astype(np.float64) ** 2, axis=1)
    b64 = np.clip(b64, -35.0, None)
    cb64 = (_C1 * b64 + _C2B / 32.0) / 4.0
    bhi = cb64.astype(np.float32).astype(_BF16)
    blo = (cb64 - bhi.astype(np.float64)).astype(np.float32).astype(_BF16)

    yt = Y.astype(_BF16).T           # [32, N]

    # A region: trains [_ND, N)
    ya = np.empty((_K, _NA), dtype=_BF16)
    ya[0:32] = yt[:, _ND:]
    ya[32] = bhi[_ND:]
    ya[33] = blo[_ND:]

    # D region: trains [0, _ND) in two halves
    ydh = np.empty((2 * _K, _ND2), dtype=_BF16)
    ydh[0:32] = yt[:, 0:_ND2]
    ydh[32] = bhi[0:_ND2]
    ydh[33] = blo[0:_ND2]
    ydh[_K:_K + 32] = yt[:, _ND2:_ND]
    ydh[_K + 32] = bhi[_ND2:_ND]
    ydh[_K + 33] = blo[_ND2:_ND]

    const = 0.5 * _DIM * np.log(2.0 * np.pi) + np.log(np.sum(w64))
    xsq = np.sum(X.astype(np.float64) ** 2, axis=1)
    dv_all = (0.5 * xsq + const)

    in_maps = []
    dvs = []
    for c in range(_NCORES):
        sl = slice(c * _QSHARD, (c + 1) * _QSHARD)
        xq = X[sl]
        xext = np.empty((_K, _QSHARD), dtype=_BF16)
        xext[0:32] = (_C1 * xq.astype(np.float64)).astype(_BF16).T
        xext[32] = np.full(_QSHARD, 4.0, dtype=_BF16)
        xext[33] = np.full(_QSHARD, 4.0, dtype=_BF16)
        in_maps.append({"ya": ya, "ydh": ydh, "xext": xext})
        dvs.append(dv_all[sl])
    return in_maps, dvs


def _gather(results, dvs):
    out = np.empty(_Q, dtype=np.float32)
    for c in range(_NCORES):
        ta = results[c]["outa"].T.reshape(_QSHARD).astype(np.float64)
        ts = results[c]["outs"].astype(np.float64).sum(axis=0)
        out[c * _QSHARD:(c + 1) * _QSHARD] = np.log(ta + ts) - dvs[c]
    return out


def kernel(X, X_train, sample_weight, _want_timing=False):
    from concourse.bass_utils import run_bass_kernel_spmd

    nc = _get_program()
    in_maps, dvs = _prep_inputs(X, X_train, sample_weight)
    kres = run_bass_kernel_spmd(
        nc, in_maps, core_ids=list(range(_NCORES)),
        trace=bool(_want_timing),
    )
    out = _gather(kres.results, dvs)
    if _want_timing:
        return out, kres
    return out
